# revision 14
# baseline (speedup 1.0000x reference)
"""Trainium2 Bass kernel for multi-head self-attention.

Problem: B=8, N=2048, C=384, H=6 heads, D=64.
  qkv = x @ qkv_w.T + qkv_b ; q,k,v split; q *= D**-0.5
  attn = softmax(q @ k.T, axis=-1); out = (attn @ v) @ proj_w.T + proj_b

Sharding: pure data-parallel, one batch element per NeuronCore (8 cores),
no collectives.

Per-core design (everything resident in SBUF, all matmuls bf16 with f32
PSUM accumulation):
  - Host pre-transposes x -> xT [C, N], weights to [in, out] layout, all
    bf16. k-bias dropped (softmax shift-invariant), v-bias folded into the
    proj bias, q-scale folded into Wq/bq.
  - q^T/k^T are stored per head with the 64 head-dims DUPLICATED onto both
    64-partition halves (q pre-halved on host so the K=128 contraction sums
    to the exact score). K=128 scores matmuls keep the PE array fully
    active; K=64 ones let the HAM activity monitor clock-gate the PE to
    1.2 GHz for the whole attention phase (measured: 462 us at K=4/8).
  - scores are computed transposed, s^T[m, q], so the softmax reduction
    (over keys m) is along partitions and can be done by a matmul: v is
    augmented per head as [v_h | ones] (even) / [ones | v_h] (odd), so ONE
    nd-matmul per e-chunk yields the numerator on the partitions the proj
    layout needs and the 64x-replicated denominator on the other half.
  - exp on ScalarE PSUM->SBUF bf16, no max-subtraction (|s| <~ 4).
  - normalize: exact DVE reciprocal of the denominator half, a SBUF->SBUF
    DMA shifts it onto the numerator partitions (engines cannot cross
    partitions; DMA can and is idle), one DVE multiply -> aT [C, N] bf16.
  - proj consumes aT as its moving operand, output written transposed
    [C, N] f32 and un-transposed on the host.
  - one shared PSUM pool with two 2-bank tag rings ("s" x2, "nd" x2 = all
    8 banks): qkv-phase tiles, scores, and proj pieces all share the "s"
    ring so early attention overlaps the prologue and proj overlaps the
    attention tail. Group (h0,qh0) defers its nd-matmuls until after the
    remaining qkv-phase work so the in-order PE queue never stalls on exp.
"""

import sys

sys.path.insert(0, "/opt/trn_rl_repo")

import numpy as np
import ml_dtypes

import concourse.bass as bass
import concourse.tile as tile
from concourse import bacc, mybir
from concourse.bass_utils import run_bass_kernel_spmd

B, N, C = 8, 2048, 384
H, D = 6, 64
SCALE = D ** -0.5
BF16 = mybir.dt.bfloat16
F32 = mybir.dt.float32
P = 128

NCORES = 8
NMT = N // P            # 16 m-tiles
QH = 1024               # q-half width for the attention inner loop

_NC = None
LAST_RESULT = None      # BassKernelResults of the most recent run


def _build_nc():
    nc = bacc.Bacc(
        "TRN2",
        target_bir_lowering=False,
        debug=False,
        enable_asserts=False,
        num_devices=NCORES,
    )

    xT_e = nc.declare_dram_parameter("xT", [C, N], BF16, isOutput=False)
    wqk_e = nc.declare_dram_parameter("wqkT", [C, 2 * C], BF16, isOutput=False)
    wv_e = nc.declare_dram_parameter("wvT", [C, C], BF16, isOutput=False)
    pw_e = nc.declare_dram_parameter("pwT", [C, C], BF16, isOutput=False)
    bq_e = nc.declare_dram_parameter("bq", [C, 1], F32, isOutput=False)
    bp_e = nc.declare_dram_parameter("bp", [C, 1], F32, isOutput=False)
    out_e = nc.declare_dram_parameter("out", [C, N], F32, isOutput=True)

    Exp = mybir.ActivationFunctionType.Exp
    Ident = mybir.ActivationFunctionType.Identity

    from contextlib import ExitStack

    with tile.TileContext(nc) as tc, ExitStack() as ctx:
        wpool = ctx.enter_context(tc.tile_pool(name="weights", bufs=1))
        xpool = ctx.enter_context(tc.tile_pool(name="xT", bufs=1))
        qkpool = ctx.enter_context(tc.tile_pool(name="qk", bufs=1))
        vpool = ctx.enter_context(tc.tile_pool(name="v", bufs=1))
        apool = ctx.enter_context(tc.tile_pool(name="aT", bufs=1))
        epool = ctx.enter_context(tc.tile_pool(name="e", bufs=18))
        rpool = ctx.enter_context(tc.tile_pool(name="r", bufs=2))
        opool = ctx.enter_context(tc.tile_pool(name="o", bufs=2))
        ps = ctx.enter_context(tc.tile_pool(name="ps", bufs=2, space="PSUM"))

        # ---- input DMAs ----
        xT = []
        for k in range(3):
            t = xpool.tile([P, N], BF16, tag=f"xT{k}", name=f"xT{k}")
            nc.sync.dma_start(out=t[:], in_=xT_e[P * k : P * (k + 1), :])
            xT.append(t)
        wqk, wv, pw = [], [], []
        for k in range(3):
            t = wpool.tile([P, 2 * C], BF16, tag=f"wqk{k}", name=f"wqk{k}")
            nc.sync.dma_start(out=t[:], in_=wqk_e[P * k : P * (k + 1), :])
            wqk.append(t)
            t = wpool.tile([P, C], BF16, tag=f"wv{k}", name=f"wv{k}")
            nc.sync.dma_start(out=t[:], in_=wv_e[P * k : P * (k + 1), :])
            wv.append(t)
            t = wpool.tile([P, C], BF16, tag=f"pw{k}", name=f"pw{k}")
            nc.sync.dma_start(out=t[:], in_=pw_e[P * k : P * (k + 1), :])
            pw.append(t)
        bq, bp = [], []
        for j in range(3):
            t = wpool.tile([P, 1], F32, tag=f"bq{j}", name=f"bq{j}")
            nc.sync.dma_start(out=t[:], in_=bq_e[P * j : P * (j + 1), :])
            bq.append(t)
            t = wpool.tile([P, 1], F32, tag=f"bp{j}", name=f"bp{j}")
            nc.sync.dma_start(out=t[:], in_=bp_e[P * j : P * (j + 1), :])
            bp.append(t)

        qdup = [qkpool.tile([P, N], BF16, tag=f"qd{m}", name=f"qd{m}") for m in range(6)]
        kdup = [qkpool.tile([P, N], BF16, tag=f"kd{m}", name=f"kd{m}") for m in range(6)]
        vaug = [
            vpool.tile([P, H * P], BF16, tag=f"va{m}", name=f"va{m}")
            for m in range(NMT)
        ]
        aT = [apool.tile([P, N], BF16, tag=f"aT{t}", name=f"aT{t}") for t in range(3)]

        # ---- qkv phase helpers ----
        def p1_mo(mo):
            # one 128-row stripe of q^T/k^T (= 2 heads' halves), in two
            # 1024-wide pieces through the shared "s" psum ring
            for half in range(2):
                piece = ps.tile([P, QH], F32, tag="s", name="qk_ps")
                for c in range(2):
                    xs = slice(QH * half + 512 * c, QH * half + 512 * (c + 1))
                    cs = slice(512 * c, 512 * (c + 1))
                    for k in range(3):
                        nc.tensor.matmul(
                            piece[:, cs],
                            wqk[k][:, P * mo : P * (mo + 1)],
                            xT[k][:, xs],
                            start=(k == 0),
                            stop=(k == 2),
                        )
                qs = slice(QH * half, QH * (half + 1))
                if mo < 3:
                    nc.vector.tensor_scalar_add(
                        qdup[2 * mo][0:64, qs], piece[0:64, :], bq[mo][0:64, :]
                    )
                    nc.vector.tensor_scalar_add(
                        qdup[2 * mo + 1][64:128, qs], piece[64:128, :],
                        bq[mo][64:128, :],
                    )
                else:
                    mk = mo - 3
                    nc.vector.tensor_copy(kdup[2 * mk][0:64, qs], piece[0:64, :])
                    nc.vector.tensor_copy(
                        kdup[2 * mk + 1][64:128, qs], piece[64:128, :]
                    )

        def dup_heads(hs):
            for hh in hs:
                if hh % 2 == 0:
                    nc.sync.dma_start(out=qdup[hh][64:128, :], in_=qdup[hh][0:64, :])
                    nc.sync.dma_start(out=kdup[hh][64:128, :], in_=kdup[hh][0:64, :])
                else:
                    nc.sync.dma_start(out=qdup[hh][0:64, :], in_=qdup[hh][64:128, :])
                    nc.sync.dma_start(out=kdup[hh][0:64, :], in_=kdup[hh][64:128, :])

        def p2_mt(mt):
            vps = ps.tile([P, C], F32, tag="nd", name="v_ps")
            for k in range(3):
                nc.tensor.matmul(
                    vps[:],
                    xT[k][:, P * mt : P * (mt + 1)],
                    wv[k][:],
                    start=(k == 0),
                    stop=(k == 2),
                )
            for h in range(H):
                off = P * h + (0 if h % 2 == 0 else D)
                nc.vector.tensor_copy(
                    vaug[mt][:, off : off + D], vps[:, D * h : D * (h + 1)]
                )

        # ---- attention helpers ----
        def emit_s_exp(h, qh, mt):
            s = ps.tile([P, QH], F32, tag="s", name="s")
            for c in range(2):
                qs = slice(QH * qh + 512 * c, QH * qh + 512 * (c + 1))
                cs = slice(512 * c, 512 * (c + 1))
                nc.tensor.matmul(
                    s[:, cs], kdup[h][:, P * mt : P * (mt + 1)], qdup[h][:, qs],
                    start=True, stop=True,
                )
            e = epool.tile([P, QH], BF16, tag="e", name="e")
            nc.scalar.activation(e[:], s[:], Exp)
            return e

        def emit_nd(h, nd, mt, e):
            for c in range(2):
                cs = slice(512 * c, 512 * (c + 1))
                nc.tensor.matmul(
                    nd[:, cs],
                    vaug[mt][:, P * h : P * (h + 1)],
                    e[:, cs],
                    start=(mt == 0), stop=(mt == NMT - 1),
                )

        def normalize(h, qh, nd):
            num_p = slice(0, 64) if h % 2 == 0 else slice(64, 128)
            den_p = slice(64, 128) if h % 2 == 0 else slice(0, 64)
            r = rpool.tile([P, QH], F32, tag="r", name="r")
            nc.vector.reciprocal(r[den_p, :], nd[den_p, :])
            nc.sync.dma_start(out=r[num_p, :], in_=r[den_p, :])
            nc.vector.tensor_mul(
                aT[h // 2][num_p, QH * qh : QH * (qh + 1)],
                nd[num_p, :],
                r[num_p, :],
            )

        def group(h, qh):
            # 1-deep software pipeline: s(mt+1) queued on PE before nd(mt)
            nd = ps.tile([P, QH], F32, tag="nd", name="nd")
            e_prev = emit_s_exp(h, qh, 0)
            for mt in range(1, NMT):
                e_cur = emit_s_exp(h, qh, mt)
                emit_nd(h, nd, mt - 1, e_prev)
                e_prev = e_cur
            emit_nd(h, nd, NMT - 1, e_prev)
            normalize(h, qh, nd)

        # ---- emission schedule ----
        # vaug ones-fill runs during the input DMAs
        for mt in range(NMT):
            nc.vector.memset(vaug[mt][:], 1.0)

        # heads 0/1 q,k stripes, then group-1 scores/exp immediately so the
        # ScalarE exp stream (the true bottleneck) starts as early as
        # possible; v and the remaining q,k stripes fill the PE slack.
        p1_mo(0)
        p1_mo(3)
        dup_heads([0, 1])

        nd0 = ps.tile([P, QH], F32, tag="nd", name="nd")
        es = [emit_s_exp(0, 0, mt) for mt in range(NMT)]
        for mt in range(NMT):
            p2_mt(mt)
        for mt in range(NMT):
            emit_nd(0, nd0, mt, es[mt])
        del es
        normalize(0, 0, nd0)

        group(0, 1)
        p1_mo(1)
        p1_mo(4)
        dup_heads([2, 3])
        group(1, 0)
        p1_mo(2)
        p1_mo(5)
        dup_heads([4, 5])
        group(1, 1)
        for h in range(2, H):
            for qh in range(2):
                group(h, qh)

        # ---- proj: out^T = pwT.T @ aT + bp, through the "s" ring ----
        for mo in range(3):
            for ph in range(2):
                pj = ps.tile([P, QH], F32, tag="s", name="pj")
                for c in range(2):
                    qs = slice(QH * ph + 512 * c, QH * ph + 512 * (c + 1))
                    cs = slice(512 * c, 512 * (c + 1))
                    for k in range(3):
                        nc.tensor.matmul(
                            pj[:, cs],
                            pw[k][:, P * mo : P * (mo + 1)],
                            aT[k][:, qs],
                            start=(k == 0),
                            stop=(k == 2),
                        )
                o = opool.tile([P, QH], F32, tag="o", name="o")
                nc.scalar.activation(o[:], pj[:], Ident, bias=bp[mo][:])
                nc.sync.dma_start(
                    out=out_e[P * mo : P * (mo + 1), QH * ph : QH * (ph + 1)],
                    in_=o[:],
                )

    nc.compile()
    return nc


def _get_nc():
    global _NC
    if _NC is None:
        _NC = _build_nc()
    return _NC


def kernel(x, qkv_w, qkv_b, proj_w, proj_b, h=None, w=None, _trace=False):
    global LAST_RESULT
    x = np.asarray(x, dtype=np.float32)
    qkv_w = np.asarray(qkv_w, dtype=np.float32)
    qkv_b = np.asarray(qkv_b, dtype=np.float32)
    proj_w = np.asarray(proj_w, dtype=np.float32)
    proj_b = np.asarray(proj_b, dtype=np.float32)

    bf16 = ml_dtypes.bfloat16
    # q scale (and the 0.5 for the duplicated-K contraction) folded into
    # Wq/bq; k-bias dropped (softmax shift-invariant); v-bias folded into
    # the proj bias (attention rows sum to 1).
    wqkT = np.concatenate(
        [qkv_w[:C] * (SCALE * 0.5), qkv_w[C : 2 * C]], axis=0
    ).T.astype(bf16).copy()                        # [C, 2C]
    wvT = qkv_w[2 * C :].T.astype(bf16).copy()     # [C, C]
    pwT = proj_w.T.astype(bf16).copy()             # [C, C]
    bq = (qkv_b[:C] * (SCALE * 0.5)).astype(np.float32).reshape(C, 1)
    bp = (proj_b + qkv_b[2 * C :] @ proj_w.T).astype(np.float32).reshape(C, 1)

    common = {"wqkT": wqkT, "wvT": wvT, "pwT": pwT, "bq": bq, "bp": bp}
    in_maps = []
    for i in range(NCORES):
        xT = np.ascontiguousarray(x[i].T).astype(bf16)
        in_maps.append({"xT": xT, **common})

    nc = _get_nc()
    import os as _os

    kw = {}
    if _os.environ.get("KEEP_TMPDIR"):
        kw["tmpdir"] = _os.environ["KEEP_TMPDIR"]
    res = run_bass_kernel_spmd(
        nc, in_maps, core_ids=list(range(NCORES)), trace=_trace, **kw
    )
    LAST_RESULT = res

    out = np.empty((B, N, C), dtype=np.float32)
    for i in range(NCORES):
        out[i] = res.results[i]["out"].T
    return out


if __name__ == "__main__":
    rng = np.random.default_rng(0)
    x = rng.standard_normal((B, N, C), dtype=np.float32)
    s = 1.0 / np.sqrt(C)
    qkv_w = rng.uniform(-s, s, (3 * C, C)).astype(np.float32)
    qkv_b = rng.uniform(-s, s, (3 * C,)).astype(np.float32)
    proj_w = rng.uniform(-s, s, (C, C)).astype(np.float32)
    proj_b = rng.uniform(-s, s, (C,)).astype(np.float32)
    out = kernel(x, qkv_w, qkv_b, proj_w, proj_b, 64, 32)
    print("out", out.shape, out.dtype, float(np.abs(out).mean()))


# revision 16
# speedup vs baseline: 1.0190x; 1.0190x over previous
"""Trainium2 Bass kernel for multi-head self-attention.

Problem: B=8, N=2048, C=384, H=6 heads, D=64.
  qkv = x @ qkv_w.T + qkv_b ; q,k,v split; q *= D**-0.5
  attn = softmax(q @ k.T, axis=-1); out = (attn @ v) @ proj_w.T + proj_b

Sharding: pure data-parallel, one batch element per NeuronCore (8 cores),
no collectives.

Per-core design (everything resident in SBUF, all matmuls bf16 with f32
PSUM accumulation):
  - Host pre-transposes x -> xT [C, N], weights to [in, out] layout, all
    bf16. k-bias dropped (softmax shift-invariant), v-bias folded into the
    proj bias, q-scale folded into Wq/bq.
  - q^T/k^T are stored per head with the 64 head-dims DUPLICATED onto both
    64-partition halves (q pre-halved on host so the K=128 contraction sums
    to the exact score). K=128 scores matmuls keep the PE array fully
    active; K=64 ones let the HAM activity monitor clock-gate the PE to
    1.2 GHz for the whole attention phase (measured: 462 us at K=4/8).
  - scores are computed transposed, s^T[m, q], so the softmax reduction
    (over keys m) is along partitions and can be done by a matmul: v is
    augmented per head as [v_h | ones] (even) / [ones | v_h] (odd), so ONE
    nd-matmul per e-chunk yields the numerator on the partitions the proj
    layout needs and the 64x-replicated denominator on the other half.
  - exp on ScalarE PSUM->SBUF bf16, no max-subtraction (|s| <~ 4).
  - normalize: exact DVE reciprocal of the denominator half, a SBUF->SBUF
    DMA shifts it onto the numerator partitions (engines cannot cross
    partitions; DMA can and is idle), one DVE multiply -> aT [C, N] bf16.
  - proj consumes aT as its moving operand, output written transposed
    [C, N] f32 and un-transposed on the host.
  - one shared PSUM pool with two 2-bank tag rings ("s" x2, "nd" x2 = all
    8 banks): qkv-phase tiles, scores, and proj pieces all share the "s"
    ring so early attention overlaps the prologue and proj overlaps the
    attention tail. Group (h0,qh0) defers its nd-matmuls until after the
    remaining qkv-phase work so the in-order PE queue never stalls on exp.
"""

import sys

sys.path.insert(0, "/opt/trn_rl_repo")

import numpy as np
import ml_dtypes

import concourse.bass as bass
import concourse.tile as tile
from concourse import bacc, mybir
from concourse.bass_utils import run_bass_kernel_spmd

B, N, C = 8, 2048, 384
H, D = 6, 64
SCALE = D ** -0.5
BF16 = mybir.dt.bfloat16
F32 = mybir.dt.float32
P = 128

NCORES = 8
NMT = N // P            # 16 m-tiles
QH = 1024               # q-half width for the attention inner loop

_NC = None
LAST_RESULT = None      # BassKernelResults of the most recent run


def _build_nc():
    nc = bacc.Bacc(
        "TRN2",
        target_bir_lowering=False,
        debug=False,
        enable_asserts=False,
        num_devices=NCORES,
    )

    xT_e = nc.declare_dram_parameter("xT", [C, N], BF16, isOutput=False)
    wqk_e = nc.declare_dram_parameter("wqkT", [C, 2 * C], BF16, isOutput=False)
    wv_e = nc.declare_dram_parameter("wvT", [C, C], BF16, isOutput=False)
    pw_e = nc.declare_dram_parameter("pwT", [C, C], BF16, isOutput=False)
    bq_e = nc.declare_dram_parameter("bq", [C, 1], F32, isOutput=False)
    bp_e = nc.declare_dram_parameter("bp", [C, 1], F32, isOutput=False)
    out_e = nc.declare_dram_parameter("out", [C, N], F32, isOutput=True)

    Exp = mybir.ActivationFunctionType.Exp
    Ident = mybir.ActivationFunctionType.Identity

    from contextlib import ExitStack

    with tile.TileContext(nc) as tc, ExitStack() as ctx:
        wpool = ctx.enter_context(tc.tile_pool(name="weights", bufs=1))
        xpool = ctx.enter_context(tc.tile_pool(name="xT", bufs=1))
        qkpool = ctx.enter_context(tc.tile_pool(name="qk", bufs=1))
        vpool = ctx.enter_context(tc.tile_pool(name="v", bufs=1))
        apool = ctx.enter_context(tc.tile_pool(name="aT", bufs=1))
        epool = ctx.enter_context(tc.tile_pool(name="e", bufs=18))
        rpool = ctx.enter_context(tc.tile_pool(name="r", bufs=2))
        opool = ctx.enter_context(tc.tile_pool(name="o", bufs=2))
        ps = ctx.enter_context(tc.tile_pool(name="ps", bufs=2, space="PSUM"))

        # ---- input DMAs ----
        xT = []
        for k in range(3):
            t = xpool.tile([P, N], BF16, tag=f"xT{k}", name=f"xT{k}")
            nc.sync.dma_start(out=t[:], in_=xT_e[P * k : P * (k + 1), :])
            xT.append(t)
        wqk, wv, pw = [], [], []
        for k in range(3):
            t = wpool.tile([P, 2 * C], BF16, tag=f"wqk{k}", name=f"wqk{k}")
            nc.scalar.dma_start(out=t[:], in_=wqk_e[P * k : P * (k + 1), :])
            wqk.append(t)
            t = wpool.tile([P, C], BF16, tag=f"wv{k}", name=f"wv{k}")
            nc.gpsimd.dma_start(out=t[:], in_=wv_e[P * k : P * (k + 1), :])
            wv.append(t)
            t = wpool.tile([P, C], BF16, tag=f"pw{k}", name=f"pw{k}")
            nc.gpsimd.dma_start(out=t[:], in_=pw_e[P * k : P * (k + 1), :])
            pw.append(t)
        bq, bp = [], []
        for j in range(3):
            t = wpool.tile([P, 1], F32, tag=f"bq{j}", name=f"bq{j}")
            nc.scalar.dma_start(out=t[:], in_=bq_e[P * j : P * (j + 1), :])
            bq.append(t)
            t = wpool.tile([P, 1], F32, tag=f"bp{j}", name=f"bp{j}")
            nc.scalar.dma_start(out=t[:], in_=bp_e[P * j : P * (j + 1), :])
            bp.append(t)

        qdup = [qkpool.tile([P, N], BF16, tag=f"qd{m}", name=f"qd{m}") for m in range(6)]
        kdup = [qkpool.tile([P, N], BF16, tag=f"kd{m}", name=f"kd{m}") for m in range(6)]
        vaug = [
            vpool.tile([P, H * P], BF16, tag=f"va{m}", name=f"va{m}")
            for m in range(NMT)
        ]
        aT = [apool.tile([P, N], BF16, tag=f"aT{t}", name=f"aT{t}") for t in range(3)]

        # ---- qkv phase helpers ----
        def p1_piece(mo, half):
            piece = ps.tile([P, QH], F32, tag="s", name="qk_ps")
            if True:
                for c in range(2):
                    xs = slice(QH * half + 512 * c, QH * half + 512 * (c + 1))
                    cs = slice(512 * c, 512 * (c + 1))
                    for k in range(3):
                        nc.tensor.matmul(
                            piece[:, cs],
                            wqk[k][:, P * mo : P * (mo + 1)],
                            xT[k][:, xs],
                            start=(k == 0),
                            stop=(k == 2),
                        )
                qs = slice(QH * half, QH * (half + 1))
                if mo < 3:
                    nc.vector.tensor_scalar_add(
                        qdup[2 * mo][0:64, qs], piece[0:64, :], bq[mo][0:64, :]
                    )
                    nc.vector.tensor_scalar_add(
                        qdup[2 * mo + 1][64:128, qs], piece[64:128, :],
                        bq[mo][64:128, :],
                    )
                else:
                    mk = mo - 3
                    nc.vector.tensor_copy(kdup[2 * mk][0:64, qs], piece[0:64, :])
                    nc.vector.tensor_copy(
                        kdup[2 * mk + 1][64:128, qs], piece[64:128, :]
                    )

        def p1_mo(mo):
            # one 128-row stripe of q^T/k^T (= 2 heads' halves), in two
            # 1024-wide pieces through the shared "s" psum ring
            p1_piece(mo, 0)
            p1_piece(mo, 1)

        def dup_heads(hs):
            for hh in hs:
                if hh % 2 == 0:
                    nc.sync.dma_start(out=qdup[hh][64:128, :], in_=qdup[hh][0:64, :])
                    nc.sync.dma_start(out=kdup[hh][64:128, :], in_=kdup[hh][0:64, :])
                else:
                    nc.sync.dma_start(out=qdup[hh][0:64, :], in_=qdup[hh][64:128, :])
                    nc.sync.dma_start(out=kdup[hh][0:64, :], in_=kdup[hh][64:128, :])

        def p2_mt(mt):
            vps = ps.tile([P, C], F32, tag="nd", name="v_ps")
            for k in range(3):
                nc.tensor.matmul(
                    vps[:],
                    xT[k][:, P * mt : P * (mt + 1)],
                    wv[k][:],
                    start=(k == 0),
                    stop=(k == 2),
                )
            for h in range(H):
                off = P * h + (0 if h % 2 == 0 else D)
                nc.vector.tensor_copy(
                    vaug[mt][:, off : off + D], vps[:, D * h : D * (h + 1)]
                )

        # ---- attention helpers ----
        def emit_s_exp(h, qh, mt):
            s = ps.tile([P, QH], F32, tag="s", name="s")
            for c in range(2):
                qs = slice(QH * qh + 512 * c, QH * qh + 512 * (c + 1))
                cs = slice(512 * c, 512 * (c + 1))
                nc.tensor.matmul(
                    s[:, cs], kdup[h][:, P * mt : P * (mt + 1)], qdup[h][:, qs],
                    start=True, stop=True,
                )
            e = epool.tile([P, QH], BF16, tag="e", name="e")
            nc.scalar.activation(e[:], s[:], Exp)
            return e

        def emit_nd(h, nd, mt, e):
            for c in range(2):
                cs = slice(512 * c, 512 * (c + 1))
                nc.tensor.matmul(
                    nd[:, cs],
                    vaug[mt][:, P * h : P * (h + 1)],
                    e[:, cs],
                    start=(mt == 0), stop=(mt == NMT - 1),
                )

        def normalize(h, qh, nd):
            num_p = slice(0, 64) if h % 2 == 0 else slice(64, 128)
            den_p = slice(64, 128) if h % 2 == 0 else slice(0, 64)
            r = rpool.tile([P, QH], F32, tag="r", name="r")
            nc.vector.reciprocal(r[den_p, :], nd[den_p, :])
            nc.sync.dma_start(out=r[num_p, :], in_=r[den_p, :])
            nc.vector.tensor_mul(
                aT[h // 2][num_p, QH * qh : QH * (qh + 1)],
                nd[num_p, :],
                r[num_p, :],
            )

        def group(h, qh, extras=()):
            # 1-deep software pipeline: s(mt+1) queued on PE before nd(mt);
            # extras are drip-fed prologue chunks filling PE/DVE slack
            extras = list(extras)
            nd = ps.tile([P, QH], F32, tag="nd", name="nd")
            e_prev = emit_s_exp(h, qh, 0)
            for mt in range(1, NMT):
                e_cur = emit_s_exp(h, qh, mt)
                emit_nd(h, nd, mt - 1, e_prev)
                e_prev = e_cur
                if mt % 3 == 0 and extras:
                    extras.pop(0)()
            emit_nd(h, nd, NMT - 1, e_prev)
            for ex in extras:
                ex()
            normalize(h, qh, nd)

        # ---- emission schedule ----
        # vaug ones-fill runs during the input DMAs
        for mt in range(NMT):
            nc.gpsimd.memset(vaug[mt][:], 1.0)

        # heads 0/1 q,k stripes, then group-1 scores/exp immediately so the
        # ScalarE exp stream (the true bottleneck) starts as early as
        # possible; v and the remaining q,k stripes fill the PE slack.
        p1_mo(0)
        p1_mo(3)
        dup_heads([0, 1])

        nd0 = ps.tile([P, QH], F32, tag="nd", name="nd")
        es = [emit_s_exp(0, 0, mt) for mt in range(NMT)]
        for mt in range(NMT):
            p2_mt(mt)
        for mt in range(NMT):
            emit_nd(0, nd0, mt, es[mt])
        del es
        normalize(0, 0, nd0)

        group(0, 1, extras=[
            lambda: p1_piece(1, 0), lambda: p1_piece(1, 1),
            lambda: p1_piece(4, 0), lambda: p1_piece(4, 1),
            lambda: dup_heads([2, 3]),
        ])
        group(1, 0, extras=[
            lambda: p1_piece(2, 0), lambda: p1_piece(2, 1),
            lambda: p1_piece(5, 0), lambda: p1_piece(5, 1),
            lambda: dup_heads([4, 5]),
        ])
        group(1, 1)
        for h in range(2, H):
            for qh in range(2):
                group(h, qh)

        # ---- proj: out^T = pwT.T @ aT + bp, through the "s" ring ----
        for mo in range(3):
            for ph in range(2):
                pj = ps.tile([P, QH], F32, tag="s", name="pj")
                for c in range(2):
                    qs = slice(QH * ph + 512 * c, QH * ph + 512 * (c + 1))
                    cs = slice(512 * c, 512 * (c + 1))
                    for k in range(3):
                        nc.tensor.matmul(
                            pj[:, cs],
                            pw[k][:, P * mo : P * (mo + 1)],
                            aT[k][:, qs],
                            start=(k == 0),
                            stop=(k == 2),
                        )
                o = opool.tile([P, QH], F32, tag="o", name="o")
                nc.scalar.activation(o[:], pj[:], Ident, bias=bp[mo][:])
                nc.sync.dma_start(
                    out=out_e[P * mo : P * (mo + 1), QH * ph : QH * (ph + 1)],
                    in_=o[:],
                )

    nc.compile()
    return nc


def _get_nc():
    global _NC
    if _NC is None:
        _NC = _build_nc()
    return _NC


def kernel(x, qkv_w, qkv_b, proj_w, proj_b, h=None, w=None, _trace=False):
    global LAST_RESULT
    x = np.asarray(x, dtype=np.float32)
    qkv_w = np.asarray(qkv_w, dtype=np.float32)
    qkv_b = np.asarray(qkv_b, dtype=np.float32)
    proj_w = np.asarray(proj_w, dtype=np.float32)
    proj_b = np.asarray(proj_b, dtype=np.float32)

    bf16 = ml_dtypes.bfloat16
    # q scale (and the 0.5 for the duplicated-K contraction) folded into
    # Wq/bq; k-bias dropped (softmax shift-invariant); v-bias folded into
    # the proj bias (attention rows sum to 1).
    wqkT = np.concatenate(
        [qkv_w[:C] * (SCALE * 0.5), qkv_w[C : 2 * C]], axis=0
    ).T.astype(bf16).copy()                        # [C, 2C]
    wvT = qkv_w[2 * C :].T.astype(bf16).copy()     # [C, C]
    pwT = proj_w.T.astype(bf16).copy()             # [C, C]
    bq = (qkv_b[:C] * (SCALE * 0.5)).astype(np.float32).reshape(C, 1)
    bp = (proj_b + qkv_b[2 * C :] @ proj_w.T).astype(np.float32).reshape(C, 1)

    common = {"wqkT": wqkT, "wvT": wvT, "pwT": pwT, "bq": bq, "bp": bp}
    in_maps = []
    for i in range(NCORES):
        xT = np.ascontiguousarray(x[i].T).astype(bf16)
        in_maps.append({"xT": xT, **common})

    nc = _get_nc()
    import os as _os

    kw = {}
    if _os.environ.get("KEEP_TMPDIR"):
        kw["tmpdir"] = _os.environ["KEEP_TMPDIR"]
    res = run_bass_kernel_spmd(
        nc, in_maps, core_ids=list(range(NCORES)), trace=_trace, **kw
    )
    LAST_RESULT = res

    out = np.empty((B, N, C), dtype=np.float32)
    for i in range(NCORES):
        out[i] = res.results[i]["out"].T
    return out


if __name__ == "__main__":
    rng = np.random.default_rng(0)
    x = rng.standard_normal((B, N, C), dtype=np.float32)
    s = 1.0 / np.sqrt(C)
    qkv_w = rng.uniform(-s, s, (3 * C, C)).astype(np.float32)
    qkv_b = rng.uniform(-s, s, (3 * C,)).astype(np.float32)
    proj_w = rng.uniform(-s, s, (C, C)).astype(np.float32)
    proj_b = rng.uniform(-s, s, (C,)).astype(np.float32)
    out = kernel(x, qkv_w, qkv_b, proj_w, proj_b, 64, 32)
    print("out", out.shape, out.dtype, float(np.abs(out).mean()))


# revision 17
# speedup vs baseline: 1.0323x; 1.0131x over previous
"""Trainium2 Bass kernel for multi-head self-attention.

Problem: B=8, N=2048, C=384, H=6 heads, D=64.
  qkv = x @ qkv_w.T + qkv_b ; q,k,v split; q *= D**-0.5
  attn = softmax(q @ k.T, axis=-1); out = (attn @ v) @ proj_w.T + proj_b

Sharding: pure data-parallel, one batch element per NeuronCore (8 cores),
no collectives.

Per-core design (everything resident in SBUF, all matmuls bf16 with f32
PSUM accumulation):
  - Host pre-transposes x -> xT [C, N], weights to [in, out] layout, all
    bf16. k-bias dropped (softmax shift-invariant), v-bias folded into the
    proj bias, q-scale folded into Wq/bq.
  - q^T/k^T are stored per head with the 64 head-dims DUPLICATED onto both
    64-partition halves (q pre-halved on host so the K=128 contraction sums
    to the exact score). K=128 scores matmuls keep the PE array fully
    active; K=64 ones let the HAM activity monitor clock-gate the PE to
    1.2 GHz for the whole attention phase (measured: 462 us at K=4/8).
  - scores are computed transposed, s^T[m, q], so the softmax reduction
    (over keys m) is along partitions and can be done by a matmul: v is
    augmented per head as [v_h | ones] (even) / [ones | v_h] (odd), so ONE
    nd-matmul per e-chunk yields the numerator on the partitions the proj
    layout needs and the 64x-replicated denominator on the other half.
  - exp on ScalarE PSUM->SBUF bf16, no max-subtraction (|s| <~ 4).
  - normalize: exact DVE reciprocal of the denominator half, a SBUF->SBUF
    DMA shifts it onto the numerator partitions (engines cannot cross
    partitions; DMA can and is idle), one DVE multiply -> aT [C, N] bf16.
  - proj consumes aT as its moving operand, output written transposed
    [C, N] f32 and un-transposed on the host.
  - one shared PSUM pool with two 2-bank tag rings ("s" x2, "nd" x2 = all
    8 banks): qkv-phase tiles, scores, and proj pieces all share the "s"
    ring so early attention overlaps the prologue and proj overlaps the
    attention tail. Group (h0,qh0) defers its nd-matmuls until after the
    remaining qkv-phase work so the in-order PE queue never stalls on exp.
"""

import sys

sys.path.insert(0, "/opt/trn_rl_repo")

import numpy as np
import ml_dtypes

import concourse.bass as bass
import concourse.tile as tile
from concourse import bacc, mybir
from concourse.bass_utils import run_bass_kernel_spmd

B, N, C = 8, 2048, 384
H, D = 6, 64
SCALE = D ** -0.5
BF16 = mybir.dt.bfloat16
F32 = mybir.dt.float32
P = 128

NCORES = 8
NMT = N // P            # 16 m-tiles
QH = 1024               # q-half width for the attention inner loop

_NC = None
LAST_RESULT = None      # BassKernelResults of the most recent run


def _build_nc():
    nc = bacc.Bacc(
        "TRN2",
        target_bir_lowering=False,
        debug=False,
        enable_asserts=False,
        num_devices=NCORES,
    )

    xT_e = nc.declare_dram_parameter("xT", [C, N], BF16, isOutput=False)
    wqk_e = nc.declare_dram_parameter("wqkT", [C, 2 * C], BF16, isOutput=False)
    wv_e = nc.declare_dram_parameter("wvT", [C, C], BF16, isOutput=False)
    pw_e = nc.declare_dram_parameter("pwT", [C, C], BF16, isOutput=False)
    bq_e = nc.declare_dram_parameter("bq", [C, 1], F32, isOutput=False)
    bp_e = nc.declare_dram_parameter("bp", [C, 1], F32, isOutput=False)
    out_e = nc.declare_dram_parameter("out", [C, N], F32, isOutput=True)

    Exp = mybir.ActivationFunctionType.Exp
    Ident = mybir.ActivationFunctionType.Identity

    from contextlib import ExitStack

    with tile.TileContext(nc) as tc, ExitStack() as ctx:
        wpool = ctx.enter_context(tc.tile_pool(name="weights", bufs=1))
        xpool = ctx.enter_context(tc.tile_pool(name="xT", bufs=1))
        qkpool = ctx.enter_context(tc.tile_pool(name="qk", bufs=1))
        vpool = ctx.enter_context(tc.tile_pool(name="v", bufs=1))
        apool = ctx.enter_context(tc.tile_pool(name="aT", bufs=1))
        epool = ctx.enter_context(tc.tile_pool(name="e", bufs=18))
        rpool = ctx.enter_context(tc.tile_pool(name="r", bufs=2))
        opool = ctx.enter_context(tc.tile_pool(name="o", bufs=2))
        ps = ctx.enter_context(tc.tile_pool(name="ps", bufs=2, space="PSUM"))

        # ---- input DMAs ----
        xT = []
        for k in range(3):
            t = xpool.tile([P, N], BF16, tag=f"xT{k}", name=f"xT{k}")
            nc.sync.dma_start(out=t[:], in_=xT_e[P * k : P * (k + 1), :])
            xT.append(t)
        wqk, wv, pw = [], [], []
        for k in range(3):
            t = wpool.tile([P, 2 * C], BF16, tag=f"wqk{k}", name=f"wqk{k}")
            nc.scalar.dma_start(out=t[:], in_=wqk_e[P * k : P * (k + 1), :])
            wqk.append(t)
            t = wpool.tile([P, C], BF16, tag=f"wv{k}", name=f"wv{k}")
            nc.gpsimd.dma_start(out=t[:], in_=wv_e[P * k : P * (k + 1), :])
            wv.append(t)
            t = wpool.tile([P, C], BF16, tag=f"pw{k}", name=f"pw{k}")
            nc.gpsimd.dma_start(out=t[:], in_=pw_e[P * k : P * (k + 1), :])
            pw.append(t)
        bq, bp = [], []
        for j in range(3):
            t = wpool.tile([P, 1], F32, tag=f"bq{j}", name=f"bq{j}")
            nc.scalar.dma_start(out=t[:], in_=bq_e[P * j : P * (j + 1), :])
            bq.append(t)
            t = wpool.tile([P, 1], F32, tag=f"bp{j}", name=f"bp{j}")
            nc.scalar.dma_start(out=t[:], in_=bp_e[P * j : P * (j + 1), :])
            bp.append(t)

        qdup = [qkpool.tile([P, N], BF16, tag=f"qd{m}", name=f"qd{m}") for m in range(6)]
        kdup = [qkpool.tile([P, N], BF16, tag=f"kd{m}", name=f"kd{m}") for m in range(6)]
        vaug = [
            vpool.tile([P, H * P], BF16, tag=f"va{m}", name=f"va{m}")
            for m in range(NMT)
        ]
        aT = [apool.tile([P, N], BF16, tag=f"aT{t}", name=f"aT{t}") for t in range(3)]

        # ---- qkv phase helpers ----
        def p1_piece(mo, half, tag="s"):
            piece = ps.tile([P, QH], F32, tag=tag, name="qk_ps")
            if True:
                for c in range(2):
                    xs = slice(QH * half + 512 * c, QH * half + 512 * (c + 1))
                    cs = slice(512 * c, 512 * (c + 1))
                    for k in range(3):
                        nc.tensor.matmul(
                            piece[:, cs],
                            wqk[k][:, P * mo : P * (mo + 1)],
                            xT[k][:, xs],
                            start=(k == 0),
                            stop=(k == 2),
                        )
                qs = slice(QH * half, QH * (half + 1))
                if mo < 3:
                    nc.vector.tensor_scalar_add(
                        qdup[2 * mo][0:64, qs], piece[0:64, :], bq[mo][0:64, :]
                    )
                    nc.vector.tensor_scalar_add(
                        qdup[2 * mo + 1][64:128, qs], piece[64:128, :],
                        bq[mo][64:128, :],
                    )
                else:
                    mk = mo - 3
                    nc.vector.tensor_copy(kdup[2 * mk][0:64, qs], piece[0:64, :])
                    nc.vector.tensor_copy(
                        kdup[2 * mk + 1][64:128, qs], piece[64:128, :]
                    )

        def p1_mo(mo):
            # one 128-row stripe of q^T/k^T (= 2 heads' halves), in two
            # 1024-wide pieces through the shared "s" psum ring
            p1_piece(mo, 0)
            p1_piece(mo, 1)

        def dup_heads(hs):
            for hh in hs:
                if hh % 2 == 0:
                    nc.sync.dma_start(out=qdup[hh][64:128, :], in_=qdup[hh][0:64, :])
                    nc.gpsimd.dma_start(out=kdup[hh][64:128, :], in_=kdup[hh][0:64, :])
                else:
                    nc.sync.dma_start(out=qdup[hh][0:64, :], in_=qdup[hh][64:128, :])
                    nc.gpsimd.dma_start(out=kdup[hh][0:64, :], in_=kdup[hh][64:128, :])

        def p2_mt(mt):
            vps = ps.tile([P, C], F32, tag="nd", name="v_ps")
            for k in range(3):
                nc.tensor.matmul(
                    vps[:],
                    xT[k][:, P * mt : P * (mt + 1)],
                    wv[k][:],
                    start=(k == 0),
                    stop=(k == 2),
                )
            for h in range(H):
                off = P * h + (0 if h % 2 == 0 else D)
                nc.vector.tensor_copy(
                    vaug[mt][:, off : off + D], vps[:, D * h : D * (h + 1)]
                )

        # ---- attention helpers ----
        def emit_s_exp(h, qh, mt):
            s = ps.tile([P, QH], F32, tag="s", name="s")
            for c in range(2):
                qs = slice(QH * qh + 512 * c, QH * qh + 512 * (c + 1))
                cs = slice(512 * c, 512 * (c + 1))
                nc.tensor.matmul(
                    s[:, cs], kdup[h][:, P * mt : P * (mt + 1)], qdup[h][:, qs],
                    start=True, stop=True,
                )
            e = epool.tile([P, QH], BF16, tag="e", name="e")
            nc.scalar.activation(e[:], s[:], Exp)
            return e

        def emit_nd(h, nd, mt, e):
            for c in range(2):
                cs = slice(512 * c, 512 * (c + 1))
                nc.tensor.matmul(
                    nd[:, cs],
                    vaug[mt][:, P * h : P * (h + 1)],
                    e[:, cs],
                    start=(mt == 0), stop=(mt == NMT - 1),
                )

        def normalize(h, qh, nd):
            num_p = slice(0, 64) if h % 2 == 0 else slice(64, 128)
            den_p = slice(64, 128) if h % 2 == 0 else slice(0, 64)
            r = rpool.tile([P, QH], F32, tag="r", name="r")
            nc.vector.reciprocal(r[den_p, :], nd[den_p, :])
            nc.sync.dma_start(out=r[num_p, :], in_=r[den_p, :])
            nc.vector.tensor_mul(
                aT[h // 2][num_p, QH * qh : QH * (qh + 1)],
                nd[num_p, :],
                r[num_p, :],
            )

        def group(h, qh, extras=()):
            # 1-deep software pipeline: s(mt+1) queued on PE before nd(mt);
            # extras are drip-fed prologue chunks filling PE/DVE slack
            extras = list(extras)
            nd = ps.tile([P, QH], F32, tag="nd", name="nd")
            e_prev = emit_s_exp(h, qh, 0)
            for mt in range(1, NMT):
                e_cur = emit_s_exp(h, qh, mt)
                emit_nd(h, nd, mt - 1, e_prev)
                e_prev = e_cur
                if mt % 3 == 0 and extras:
                    extras.pop(0)()
            emit_nd(h, nd, NMT - 1, e_prev)
            for ex in extras:
                ex()
            normalize(h, qh, nd)

        # ---- emission schedule ----
        # vaug ones-fill runs during the input DMAs
        for mt in range(NMT):
            nc.gpsimd.memset(vaug[mt][:], 1.0)

        # minimal prologue: exactly what group (h0, qh0) needs — the full
        # k stripe of heads 0/1 and the qh0 half of their q stripe — so the
        # ScalarE exp stream (the true bottleneck) starts as early as
        # possible. Everything else fills PE/DVE slack inside the groups.
        p1_piece(0, 0)
        p1_piece(3, 0)
        p1_piece(3, 1)
        nc.sync.dma_start(out=qdup[0][64:128, 0:QH], in_=qdup[0][0:64, 0:QH])
        nc.gpsimd.dma_start(out=kdup[0][64:128, :], in_=kdup[0][0:64, :])

        es = [emit_s_exp(0, 0, mt) for mt in range(NMT)]
        for mt in range(NMT):
            p2_mt(mt)
        nd0 = ps.tile([P, QH], F32, tag="nd", name="nd")
        p1_piece(0, 1, tag="nd")
        nc.sync.dma_start(out=qdup[0][64:128, QH:N], in_=qdup[0][0:64, QH:N])
        nc.sync.dma_start(out=qdup[1][0:64, :], in_=qdup[1][64:128, :])
        nc.gpsimd.dma_start(out=kdup[1][0:64, :], in_=kdup[1][64:128, :])
        for mt in range(NMT):
            emit_nd(0, nd0, mt, es[mt])
        del es
        normalize(0, 0, nd0)

        group(0, 1, extras=[
            lambda: p1_piece(1, 0, tag="nd"), lambda: p1_piece(1, 1, tag="nd"),
            lambda: p1_piece(4, 0, tag="nd"), lambda: p1_piece(4, 1, tag="nd"),
            lambda: dup_heads([2, 3]),
        ])
        group(1, 0, extras=[
            lambda: p1_piece(2, 0, tag="nd"), lambda: p1_piece(2, 1, tag="nd"),
            lambda: p1_piece(5, 0, tag="nd"), lambda: p1_piece(5, 1, tag="nd"),
            lambda: dup_heads([4, 5]),
        ])
        group(1, 1)
        for h in range(2, H):
            for qh in range(2):
                group(h, qh)

        # ---- proj: out^T = pwT.T @ aT + bp, through the "s" ring ----
        for mo in range(3):
            for ph in range(2):
                pj = ps.tile([P, QH], F32, tag="s", name="pj")
                for c in range(2):
                    qs = slice(QH * ph + 512 * c, QH * ph + 512 * (c + 1))
                    cs = slice(512 * c, 512 * (c + 1))
                    for k in range(3):
                        nc.tensor.matmul(
                            pj[:, cs],
                            pw[k][:, P * mo : P * (mo + 1)],
                            aT[k][:, qs],
                            start=(k == 0),
                            stop=(k == 2),
                        )
                o = opool.tile([P, QH], F32, tag="o", name="o")
                nc.scalar.activation(o[:], pj[:], Ident, bias=bp[mo][:])
                eng = [nc.sync, nc.gpsimd, nc.scalar][(2 * mo + ph) % 3]
                eng.dma_start(
                    out=out_e[P * mo : P * (mo + 1), QH * ph : QH * (ph + 1)],
                    in_=o[:],
                )

    nc.compile()
    return nc


def _get_nc():
    global _NC
    if _NC is None:
        _NC = _build_nc()
    return _NC


def kernel(x, qkv_w, qkv_b, proj_w, proj_b, h=None, w=None, _trace=False):
    global LAST_RESULT
    x = np.asarray(x, dtype=np.float32)
    qkv_w = np.asarray(qkv_w, dtype=np.float32)
    qkv_b = np.asarray(qkv_b, dtype=np.float32)
    proj_w = np.asarray(proj_w, dtype=np.float32)
    proj_b = np.asarray(proj_b, dtype=np.float32)

    bf16 = ml_dtypes.bfloat16
    # q scale (and the 0.5 for the duplicated-K contraction) folded into
    # Wq/bq; k-bias dropped (softmax shift-invariant); v-bias folded into
    # the proj bias (attention rows sum to 1).
    wqkT = np.concatenate(
        [qkv_w[:C] * (SCALE * 0.5), qkv_w[C : 2 * C]], axis=0
    ).T.astype(bf16).copy()                        # [C, 2C]
    wvT = qkv_w[2 * C :].T.astype(bf16).copy()     # [C, C]
    pwT = proj_w.T.astype(bf16).copy()             # [C, C]
    bq = (qkv_b[:C] * (SCALE * 0.5)).astype(np.float32).reshape(C, 1)
    bp = (proj_b + qkv_b[2 * C :] @ proj_w.T).astype(np.float32).reshape(C, 1)

    common = {"wqkT": wqkT, "wvT": wvT, "pwT": pwT, "bq": bq, "bp": bp}
    in_maps = []
    for i in range(NCORES):
        xT = np.ascontiguousarray(x[i].T).astype(bf16)
        in_maps.append({"xT": xT, **common})

    nc = _get_nc()
    import os as _os

    kw = {}
    if _os.environ.get("KEEP_TMPDIR"):
        kw["tmpdir"] = _os.environ["KEEP_TMPDIR"]
    res = run_bass_kernel_spmd(
        nc, in_maps, core_ids=list(range(NCORES)), trace=_trace, **kw
    )
    LAST_RESULT = res

    out = np.empty((B, N, C), dtype=np.float32)
    for i in range(NCORES):
        out[i] = res.results[i]["out"].T
    return out


if __name__ == "__main__":
    rng = np.random.default_rng(0)
    x = rng.standard_normal((B, N, C), dtype=np.float32)
    s = 1.0 / np.sqrt(C)
    qkv_w = rng.uniform(-s, s, (3 * C, C)).astype(np.float32)
    qkv_b = rng.uniform(-s, s, (3 * C,)).astype(np.float32)
    proj_w = rng.uniform(-s, s, (C, C)).astype(np.float32)
    proj_b = rng.uniform(-s, s, (C,)).astype(np.float32)
    out = kernel(x, qkv_w, qkv_b, proj_w, proj_b, 64, 32)
    print("out", out.shape, out.dtype, float(np.abs(out).mean()))


# revision 18
# speedup vs baseline: 1.1013x; 1.0669x over previous
"""Trainium2 Bass kernel for multi-head self-attention.

Problem: B=8, N=2048, C=384, H=6 heads, D=64.
  qkv = x @ qkv_w.T + qkv_b ; q,k,v split; q *= D**-0.5
  attn = softmax(q @ k.T, axis=-1); out = (attn @ v) @ proj_w.T + proj_b

Sharding: pure data-parallel, one batch element per NeuronCore (8 cores),
no collectives.

Per-core design (everything resident in SBUF, all matmuls bf16 with f32
PSUM accumulation):
  - Host pre-transposes x -> xT [C, N], weights to [in, out] layout, all
    bf16. k-bias dropped (softmax shift-invariant), v-bias folded into the
    proj bias, q-scale folded into Wq/bq.
  - q^T/k^T are stored per head with the 64 head-dims DUPLICATED onto both
    64-partition halves (q pre-halved on host so the K=128 contraction sums
    to the exact score). K=128 scores matmuls keep the PE array fully
    active; K=64 ones let the HAM activity monitor clock-gate the PE to
    1.2 GHz for the whole attention phase (measured: 462 us at K=4/8).
  - scores are computed transposed, s^T[m, q], so the softmax reduction
    (over keys m) is along partitions and can be done by a matmul: v is
    augmented per head as [v_h | ones] (even) / [ones | v_h] (odd), so ONE
    nd-matmul per e-chunk yields the numerator on the partitions the proj
    layout needs and the 64x-replicated denominator on the other half.
  - exp on ScalarE PSUM->SBUF bf16, no max-subtraction (|s| <~ 4).
  - normalize: exact DVE reciprocal of the denominator half, a SBUF->SBUF
    DMA shifts it onto the numerator partitions (engines cannot cross
    partitions; DMA can and is idle), one DVE multiply -> aT [C, N] bf16.
  - proj consumes aT as its moving operand, output written transposed
    [C, N] f32 and un-transposed on the host.
  - one shared PSUM pool with two 2-bank tag rings ("s" x2, "nd" x2 = all
    8 banks): qkv-phase tiles, scores, and proj pieces all share the "s"
    ring so early attention overlaps the prologue and proj overlaps the
    attention tail. Group (h0,qh0) defers its nd-matmuls until after the
    remaining qkv-phase work so the in-order PE queue never stalls on exp.
"""

import sys

sys.path.insert(0, "/opt/trn_rl_repo")

import numpy as np
import ml_dtypes

import concourse.bass as bass
import concourse.tile as tile
from concourse import bacc, mybir
from concourse.bass_utils import run_bass_kernel_spmd

B, N, C = 8, 2048, 384
H, D = 6, 64
SCALE = D ** -0.5
BF16 = mybir.dt.bfloat16
F32 = mybir.dt.float32
P = 128

NCORES = 8
NMT = N // P            # 16 m-tiles
QH = 1024               # q-half width for the attention inner loop

_NC = None
LAST_RESULT = None      # BassKernelResults of the most recent run


def _build_nc():
    nc = bacc.Bacc(
        "TRN2",
        target_bir_lowering=False,
        debug=False,
        enable_asserts=False,
        num_devices=NCORES,
    )

    xT_e = nc.declare_dram_parameter("xT", [C, N], BF16, isOutput=False)
    wqk_e = nc.declare_dram_parameter("wqkT", [C, 2 * C], BF16, isOutput=False)
    wv_e = nc.declare_dram_parameter("wvT", [C, C], BF16, isOutput=False)
    pw_e = nc.declare_dram_parameter("pwT", [C, C], BF16, isOutput=False)
    bq_e = nc.declare_dram_parameter("bq", [C, 1], F32, isOutput=False)
    bp_e = nc.declare_dram_parameter("bp", [C, 1], F32, isOutput=False)
    ones_e = nc.declare_dram_parameter("vones", [P, H * P], BF16, isOutput=False)
    out_e = nc.declare_dram_parameter("out", [C, N], F32, isOutput=True)

    Exp = mybir.ActivationFunctionType.Exp
    Ident = mybir.ActivationFunctionType.Identity

    from contextlib import ExitStack

    with tile.TileContext(nc) as tc, ExitStack() as ctx:
        wpool = ctx.enter_context(tc.tile_pool(name="weights", bufs=1))
        xpool = ctx.enter_context(tc.tile_pool(name="xT", bufs=1))
        qkpool = ctx.enter_context(tc.tile_pool(name="qk", bufs=1))
        vpool = ctx.enter_context(tc.tile_pool(name="v", bufs=1))
        apool = ctx.enter_context(tc.tile_pool(name="aT", bufs=1))
        epool = ctx.enter_context(tc.tile_pool(name="e", bufs=18))
        rpool = ctx.enter_context(tc.tile_pool(name="r", bufs=2))
        opool = ctx.enter_context(tc.tile_pool(name="o", bufs=2))
        ps = ctx.enter_context(tc.tile_pool(name="ps", bufs=2, space="PSUM"))

        # ---- input DMAs ----
        xT = []
        for k in range(3):
            t = xpool.tile([P, N], BF16, tag=f"xT{k}", name=f"xT{k}")
            nc.sync.dma_start(out=t[:], in_=xT_e[P * k : P * (k + 1), :])
            xT.append(t)
        wqk, wv, pw = [], [], []
        for k in range(3):
            t = wpool.tile([P, 2 * C], BF16, tag=f"wqk{k}", name=f"wqk{k}")
            nc.scalar.dma_start(out=t[:], in_=wqk_e[P * k : P * (k + 1), :])
            wqk.append(t)
            t = wpool.tile([P, C], BF16, tag=f"wv{k}", name=f"wv{k}")
            nc.gpsimd.dma_start(out=t[:], in_=wv_e[P * k : P * (k + 1), :])
            wv.append(t)
            t = wpool.tile([P, C], BF16, tag=f"pw{k}", name=f"pw{k}")
            nc.gpsimd.dma_start(out=t[:], in_=pw_e[P * k : P * (k + 1), :])
            pw.append(t)
        bq, bp = [], []
        for j in range(3):
            t = wpool.tile([P, 1], F32, tag=f"bq{j}", name=f"bq{j}")
            nc.scalar.dma_start(out=t[:], in_=bq_e[P * j : P * (j + 1), :])
            bq.append(t)
            t = wpool.tile([P, 1], F32, tag=f"bp{j}", name=f"bp{j}")
            nc.scalar.dma_start(out=t[:], in_=bp_e[P * j : P * (j + 1), :])
            bp.append(t)

        qdup = [qkpool.tile([P, N], BF16, tag=f"qd{m}", name=f"qd{m}") for m in range(6)]
        kdup = [qkpool.tile([P, N], BF16, tag=f"kd{m}", name=f"kd{m}") for m in range(6)]
        vaug = [
            vpool.tile([P, H * P], BF16, tag=f"va{m}", name=f"va{m}")
            for m in range(NMT)
        ]
        aT = [apool.tile([P, N], BF16, tag=f"aT{t}", name=f"aT{t}") for t in range(3)]

        # ---- qkv phase helpers ----
        def p1_piece(mo, half, tag="s"):
            piece = ps.tile([P, QH], F32, tag=tag, name="qk_ps")
            if True:
                for c in range(2):
                    xs = slice(QH * half + 512 * c, QH * half + 512 * (c + 1))
                    cs = slice(512 * c, 512 * (c + 1))
                    for k in range(3):
                        nc.tensor.matmul(
                            piece[:, cs],
                            wqk[k][:, P * mo : P * (mo + 1)],
                            xT[k][:, xs],
                            start=(k == 0),
                            stop=(k == 2),
                        )
                qs = slice(QH * half, QH * (half + 1))
                if mo < 3:
                    nc.vector.tensor_scalar_add(
                        qdup[2 * mo][0:64, qs], piece[0:64, :], bq[mo][0:64, :]
                    )
                    nc.vector.tensor_scalar_add(
                        qdup[2 * mo + 1][64:128, qs], piece[64:128, :],
                        bq[mo][64:128, :],
                    )
                else:
                    mk = mo - 3
                    nc.vector.tensor_copy(kdup[2 * mk][0:64, qs], piece[0:64, :])
                    nc.vector.tensor_copy(
                        kdup[2 * mk + 1][64:128, qs], piece[64:128, :]
                    )

        def p1_mo(mo):
            # one 128-row stripe of q^T/k^T (= 2 heads' halves), in two
            # 1024-wide pieces through the shared "s" psum ring
            p1_piece(mo, 0)
            p1_piece(mo, 1)

        def dup_heads(hs):
            for hh in hs:
                if hh % 2 == 0:
                    nc.sync.dma_start(out=qdup[hh][64:128, :], in_=qdup[hh][0:64, :])
                    nc.gpsimd.dma_start(out=kdup[hh][64:128, :], in_=kdup[hh][0:64, :])
                else:
                    nc.sync.dma_start(out=qdup[hh][0:64, :], in_=qdup[hh][64:128, :])
                    nc.gpsimd.dma_start(out=kdup[hh][0:64, :], in_=kdup[hh][64:128, :])

        def p2_mt(mt):
            vps = ps.tile([P, C], F32, tag="nd", name="v_ps")
            for k in range(3):
                nc.tensor.matmul(
                    vps[:],
                    xT[k][:, P * mt : P * (mt + 1)],
                    wv[k][:],
                    start=(k == 0),
                    stop=(k == 2),
                )
            # even heads' v -> cols 256a+0, odd heads' -> 256a+192,
            # via two strided casts (ones blocks pre-filled by DMA)
            va = vaug[mt].rearrange("p (a b d) -> p a b d", a=3, b=4, d=D)
            vp = vps.rearrange("p (a c d) -> p a c d", a=3, c=2, d=D)
            nc.vector.tensor_copy(va[:, :, 0, :], vp[:, :, 0, :])
            nc.vector.tensor_copy(va[:, :, 3, :], vp[:, :, 1, :])

        # ---- attention helpers ----
        def emit_s_exp(h, qh, mt):
            s = ps.tile([P, QH], F32, tag="s", name="s")
            for c in range(2):
                qs = slice(QH * qh + 512 * c, QH * qh + 512 * (c + 1))
                cs = slice(512 * c, 512 * (c + 1))
                nc.tensor.matmul(
                    s[:, cs], kdup[h][:, P * mt : P * (mt + 1)], qdup[h][:, qs],
                    start=True, stop=True,
                )
            e = epool.tile([P, QH], BF16, tag="e", name="e")
            nc.scalar.activation(e[:], s[:], Exp)
            return e

        def emit_nd(h, nd, mt, e):
            for c in range(2):
                cs = slice(512 * c, 512 * (c + 1))
                nc.tensor.matmul(
                    nd[:, cs],
                    vaug[mt][:, P * h : P * (h + 1)],
                    e[:, cs],
                    start=(mt == 0), stop=(mt == NMT - 1),
                )

        def normalize(h, qh, nd):
            num_p = slice(0, 64) if h % 2 == 0 else slice(64, 128)
            den_p = slice(64, 128) if h % 2 == 0 else slice(0, 64)
            r = rpool.tile([P, QH], F32, tag="r", name="r")
            nc.vector.reciprocal(r[den_p, :], nd[den_p, :])
            nc.sync.dma_start(out=r[num_p, :], in_=r[den_p, :])
            nc.vector.tensor_mul(
                aT[h // 2][num_p, QH * qh : QH * (qh + 1)],
                nd[num_p, :],
                r[num_p, :],
            )

        def group(h, qh, extras=()):
            # 1-deep software pipeline: s(mt+1) queued on PE before nd(mt);
            # extras are drip-fed prologue chunks filling PE/DVE slack
            extras = list(extras)
            nd = ps.tile([P, QH], F32, tag="nd", name="nd")
            e_prev = emit_s_exp(h, qh, 0)
            for mt in range(1, NMT):
                e_cur = emit_s_exp(h, qh, mt)
                emit_nd(h, nd, mt - 1, e_prev)
                e_prev = e_cur
                if mt % 3 == 0 and extras:
                    extras.pop(0)()
            emit_nd(h, nd, NMT - 1, e_prev)
            for ex in extras:
                ex()
            normalize(h, qh, nd)

        # ---- emission schedule ----
        # vaug ones pattern arrives by DMA (v slots overwritten by p2 casts)
        for mt in range(NMT):
            nc.gpsimd.dma_start(out=vaug[mt][:], in_=ones_e[:])

        # minimal critical prologue for the first scores: q stripe half 0
        # and k stripe m-tiles 0..7 of heads 0/1, dups on the idle sync queue
        p1_piece(0, 0)
        p1_piece(3, 0)
        nc.sync.dma_start(out=qdup[0][64:128, 0:QH], in_=qdup[0][0:64, 0:QH])
        nc.sync.dma_start(out=kdup[0][64:128, 0:QH], in_=kdup[0][0:64, 0:QH])

        es0 = [emit_s_exp(0, 0, mt) for mt in range(8)]
        p1_piece(3, 1, tag="nd")
        nc.sync.dma_start(out=kdup[0][64:128, QH:N], in_=kdup[0][0:64, QH:N])
        es0 += [emit_s_exp(0, 0, mt) for mt in range(8, NMT)]

        for mt in range(NMT):
            p2_mt(mt)

        p1_piece(0, 1, tag="nd")
        nc.sync.dma_start(out=qdup[0][64:128, QH:N], in_=qdup[0][0:64, QH:N])
        nc.sync.dma_start(out=qdup[1][0:64, :], in_=qdup[1][64:128, :])
        nc.gpsimd.dma_start(out=kdup[1][0:64, :], in_=kdup[1][64:128, :])

        # global 1-group-deep pipeline: group g's nd-matmuls interleave with
        # group g+1's scores/exp so the PE queue never drains at boundaries
        seq = [(h, qh) for h in range(H) for qh in range(2)]
        extras_map = {
            2: [lambda: p1_piece(1, 0, tag="nd"), lambda: p1_piece(1, 1, tag="nd")],
            3: [lambda: p1_piece(4, 0, tag="nd"), lambda: p1_piece(4, 1, tag="nd"),
                lambda: dup_heads([2, 3])],
            5: [lambda: p1_piece(2, 0, tag="nd"), lambda: p1_piece(2, 1, tag="nd")],
            6: [lambda: p1_piece(5, 0, tag="nd"), lambda: p1_piece(5, 1, tag="nd"),
                lambda: dup_heads([4, 5])],
        }
        es_prev = es0
        nd_prev = ps.tile([P, QH], F32, tag="nd", name="nd")
        hq_prev = (0, 0)
        for gi in range(1, len(seq)):
            h, qh = seq[gi]
            extras = list(extras_map.get(gi, ()))
            if gi == len(seq) - 1:
                # last group: intra-group pipeline so its nd finishes with
                # its scores and the kernel tail stays short
                for mt in range(NMT):
                    emit_nd(hq_prev[0], nd_prev, mt, es_prev[mt])
                normalize(hq_prev[0], hq_prev[1], nd_prev)
                nd = ps.tile([P, QH], F32, tag="nd", name="nd")
                e_prev = emit_s_exp(h, qh, 0)
                for mt in range(1, NMT):
                    e_cur = emit_s_exp(h, qh, mt)
                    emit_nd(h, nd, mt - 1, e_prev)
                    e_prev = e_cur
                emit_nd(h, nd, NMT - 1, e_prev)
                normalize(h, qh, nd)
                break
            es_cur = []
            nd_cur = ps.tile([P, QH], F32, tag="nd", name="nd")
            for mt in range(NMT):
                es_cur.append(emit_s_exp(h, qh, mt))
                emit_nd(hq_prev[0], nd_prev, mt, es_prev[mt])
                if mt % 4 == 3 and extras:
                    extras.pop(0)()
            for ex in extras:
                ex()
            normalize(hq_prev[0], hq_prev[1], nd_prev)
            es_prev, nd_prev, hq_prev = es_cur, nd_cur, (h, qh)

        # ---- proj: out^T = pwT.T @ aT + bp, through the "s" ring ----
        for mo in range(3):
            for ph in range(2):
                pj = ps.tile([P, QH], F32, tag="s", name="pj")
                for c in range(2):
                    qs = slice(QH * ph + 512 * c, QH * ph + 512 * (c + 1))
                    cs = slice(512 * c, 512 * (c + 1))
                    for k in range(3):
                        nc.tensor.matmul(
                            pj[:, cs],
                            pw[k][:, P * mo : P * (mo + 1)],
                            aT[k][:, qs],
                            start=(k == 0),
                            stop=(k == 2),
                        )
                o = opool.tile([P, QH], F32, tag="o", name="o")
                nc.scalar.activation(o[:], pj[:], Ident, bias=bp[mo][:])
                eng = [nc.sync, nc.gpsimd, nc.scalar][(2 * mo + ph) % 3]
                eng.dma_start(
                    out=out_e[P * mo : P * (mo + 1), QH * ph : QH * (ph + 1)],
                    in_=o[:],
                )

    nc.compile()
    return nc


def _get_nc():
    global _NC
    if _NC is None:
        _NC = _build_nc()
    return _NC


def kernel(x, qkv_w, qkv_b, proj_w, proj_b, h=None, w=None, _trace=False):
    global LAST_RESULT
    x = np.asarray(x, dtype=np.float32)
    qkv_w = np.asarray(qkv_w, dtype=np.float32)
    qkv_b = np.asarray(qkv_b, dtype=np.float32)
    proj_w = np.asarray(proj_w, dtype=np.float32)
    proj_b = np.asarray(proj_b, dtype=np.float32)

    bf16 = ml_dtypes.bfloat16
    # q scale (and the 0.5 for the duplicated-K contraction) folded into
    # Wq/bq; k-bias dropped (softmax shift-invariant); v-bias folded into
    # the proj bias (attention rows sum to 1).
    wqkT = np.concatenate(
        [qkv_w[:C] * (SCALE * 0.5), qkv_w[C : 2 * C]], axis=0
    ).T.astype(bf16).copy()                        # [C, 2C]
    wvT = qkv_w[2 * C :].T.astype(bf16).copy()     # [C, C]
    pwT = proj_w.T.astype(bf16).copy()             # [C, C]
    bq = (qkv_b[:C] * (SCALE * 0.5)).astype(np.float32).reshape(C, 1)
    bp = (proj_b + qkv_b[2 * C :] @ proj_w.T).astype(np.float32).reshape(C, 1)

    vones = np.ones((P, H * P), dtype=bf16)
    common = {"wqkT": wqkT, "wvT": wvT, "pwT": pwT, "bq": bq, "bp": bp,
              "vones": vones}
    in_maps = []
    for i in range(NCORES):
        xT = np.ascontiguousarray(x[i].T).astype(bf16)
        in_maps.append({"xT": xT, **common})

    nc = _get_nc()
    import os as _os

    kw = {}
    if _os.environ.get("KEEP_TMPDIR"):
        kw["tmpdir"] = _os.environ["KEEP_TMPDIR"]
    res = run_bass_kernel_spmd(
        nc, in_maps, core_ids=list(range(NCORES)), trace=_trace, **kw
    )
    LAST_RESULT = res

    out = np.empty((B, N, C), dtype=np.float32)
    for i in range(NCORES):
        out[i] = res.results[i]["out"].T
    return out


if __name__ == "__main__":
    rng = np.random.default_rng(0)
    x = rng.standard_normal((B, N, C), dtype=np.float32)
    s = 1.0 / np.sqrt(C)
    qkv_w = rng.uniform(-s, s, (3 * C, C)).astype(np.float32)
    qkv_b = rng.uniform(-s, s, (3 * C,)).astype(np.float32)
    proj_w = rng.uniform(-s, s, (C, C)).astype(np.float32)
    proj_b = rng.uniform(-s, s, (C,)).astype(np.float32)
    out = kernel(x, qkv_w, qkv_b, proj_w, proj_b, 64, 32)
    print("out", out.shape, out.dtype, float(np.abs(out).mean()))


# revision 19
# speedup vs baseline: 1.1098x; 1.0077x over previous
"""Trainium2 Bass kernel for multi-head self-attention.

Problem: B=8, N=2048, C=384, H=6 heads, D=64.
  qkv = x @ qkv_w.T + qkv_b ; q,k,v split; q *= D**-0.5
  attn = softmax(q @ k.T, axis=-1); out = (attn @ v) @ proj_w.T + proj_b

Sharding: pure data-parallel, one batch element per NeuronCore (8 cores),
no collectives.

Per-core design (everything resident in SBUF, all matmuls bf16 with f32
PSUM accumulation):
  - Host pre-transposes x -> xT [C, N], weights to [in, out] layout, all
    bf16. k-bias dropped (softmax shift-invariant), v-bias folded into the
    proj bias, q-scale folded into Wq/bq.
  - q^T/k^T are stored per head with the 64 head-dims DUPLICATED onto both
    64-partition halves (q pre-halved on host so the K=128 contraction sums
    to the exact score). K=128 scores matmuls keep the PE array fully
    active; K=64 ones let the HAM activity monitor clock-gate the PE to
    1.2 GHz for the whole attention phase (measured: 462 us at K=4/8).
  - scores are computed transposed, s^T[m, q], so the softmax reduction
    (over keys m) is along partitions and can be done by a matmul: v is
    augmented per head as [v_h | ones] (even) / [ones | v_h] (odd), so ONE
    nd-matmul per e-chunk yields the numerator on the partitions the proj
    layout needs and the 64x-replicated denominator on the other half.
  - exp on ScalarE PSUM->SBUF bf16, no max-subtraction (|s| <~ 4).
  - normalize: exact DVE reciprocal of the denominator half, a SBUF->SBUF
    DMA shifts it onto the numerator partitions (engines cannot cross
    partitions; DMA can and is idle), one DVE multiply -> aT [C, N] bf16.
  - proj consumes aT as its moving operand, output written transposed
    [C, N] f32 and un-transposed on the host.
  - one shared PSUM pool with two 2-bank tag rings ("s" x2, "nd" x2 = all
    8 banks): qkv-phase tiles, scores, and proj pieces all share the "s"
    ring so early attention overlaps the prologue and proj overlaps the
    attention tail. Group (h0,qh0) defers its nd-matmuls until after the
    remaining qkv-phase work so the in-order PE queue never stalls on exp.
"""

import sys

sys.path.insert(0, "/opt/trn_rl_repo")

import numpy as np
import ml_dtypes

import concourse.bass as bass
import concourse.tile as tile
from concourse import bacc, mybir
from concourse.bass_utils import run_bass_kernel_spmd

B, N, C = 8, 2048, 384
H, D = 6, 64
SCALE = D ** -0.5
BF16 = mybir.dt.bfloat16
F32 = mybir.dt.float32
P = 128

NCORES = 8
NMT = N // P            # 16 m-tiles
QH = 1024               # q-half width for the attention inner loop

_NC = None
LAST_RESULT = None      # BassKernelResults of the most recent run


def _build_nc():
    nc = bacc.Bacc(
        "TRN2",
        target_bir_lowering=False,
        debug=False,
        enable_asserts=False,
        num_devices=NCORES,
    )

    xT_e = nc.declare_dram_parameter("xT", [C, N], BF16, isOutput=False)
    wqk_e = nc.declare_dram_parameter("wqkT", [C, 2 * C], BF16, isOutput=False)
    wv_e = nc.declare_dram_parameter("wvT", [C, C], BF16, isOutput=False)
    pw_e = nc.declare_dram_parameter("pwT", [C, C], BF16, isOutput=False)
    bq_e = nc.declare_dram_parameter("bq", [C, 1], F32, isOutput=False)
    bp_e = nc.declare_dram_parameter("bp", [C, 1], F32, isOutput=False)
    ones_e = nc.declare_dram_parameter("vones", [P, H * P], BF16, isOutput=False)
    out_e = nc.declare_dram_parameter("out", [C, N], F32, isOutput=True)

    Exp = mybir.ActivationFunctionType.Exp
    Ident = mybir.ActivationFunctionType.Identity

    from contextlib import ExitStack

    with tile.TileContext(nc) as tc, ExitStack() as ctx:
        wpool = ctx.enter_context(tc.tile_pool(name="weights", bufs=1))
        xpool = ctx.enter_context(tc.tile_pool(name="xT", bufs=1))
        qkpool = ctx.enter_context(tc.tile_pool(name="qk", bufs=1))
        vpool = ctx.enter_context(tc.tile_pool(name="v", bufs=1))
        apool = ctx.enter_context(tc.tile_pool(name="aT", bufs=1))
        epool = ctx.enter_context(tc.tile_pool(name="e", bufs=24))
        rpool = ctx.enter_context(tc.tile_pool(name="r", bufs=2))
        opool = ctx.enter_context(tc.tile_pool(name="o", bufs=2))
        ps = ctx.enter_context(tc.tile_pool(name="ps", bufs=2, space="PSUM"))

        # ---- input DMAs ----
        xT = []
        for k, eng in zip(range(3), [nc.sync, nc.gpsimd, nc.scalar]):
            t = xpool.tile([P, N], BF16, tag=f"xT{k}", name=f"xT{k}")
            eng.dma_start(out=t[:], in_=xT_e[P * k : P * (k + 1), :])
            xT.append(t)
        wqk, wv, pw = [], [], []
        for k in range(3):
            t = wpool.tile([P, 2 * C], BF16, tag=f"wqk{k}", name=f"wqk{k}")
            nc.scalar.dma_start(out=t[:], in_=wqk_e[P * k : P * (k + 1), :])
            wqk.append(t)
            t = wpool.tile([P, C], BF16, tag=f"wv{k}", name=f"wv{k}")
            nc.gpsimd.dma_start(out=t[:], in_=wv_e[P * k : P * (k + 1), :])
            wv.append(t)
            t = wpool.tile([P, C], BF16, tag=f"pw{k}", name=f"pw{k}")
            nc.gpsimd.dma_start(out=t[:], in_=pw_e[P * k : P * (k + 1), :])
            pw.append(t)
        bq, bp = [], []
        for j in range(3):
            t = wpool.tile([P, 1], F32, tag=f"bq{j}", name=f"bq{j}")
            nc.scalar.dma_start(out=t[:], in_=bq_e[P * j : P * (j + 1), :])
            bq.append(t)
            t = wpool.tile([P, 1], F32, tag=f"bp{j}", name=f"bp{j}")
            nc.scalar.dma_start(out=t[:], in_=bp_e[P * j : P * (j + 1), :])
            bp.append(t)

        qdup = [qkpool.tile([P, N], BF16, tag=f"qd{m}", name=f"qd{m}") for m in range(6)]
        kdup = [qkpool.tile([P, N], BF16, tag=f"kd{m}", name=f"kd{m}") for m in range(6)]
        vaug = [
            vpool.tile([P, H * P], BF16, tag=f"va{m}", name=f"va{m}")
            for m in range(NMT)
        ]
        aT = [apool.tile([P, N], BF16, tag=f"aT{t}", name=f"aT{t}") for t in range(3)]

        # ---- qkv phase helpers ----
        def p1_piece(mo, half, tag="s", act_copy=False):
            piece = ps.tile([P, QH], F32, tag=tag, name="qk_ps")
            if True:
                for c in range(2):
                    xs = slice(QH * half + 512 * c, QH * half + 512 * (c + 1))
                    cs = slice(512 * c, 512 * (c + 1))
                    for k in range(3):
                        nc.tensor.matmul(
                            piece[:, cs],
                            wqk[k][:, P * mo : P * (mo + 1)],
                            xT[k][:, xs],
                            start=(k == 0),
                            stop=(k == 2),
                        )
                qs = slice(QH * half, QH * (half + 1))
                if mo < 3:
                    if act_copy:
                        nc.scalar.activation(
                            qdup[2 * mo][0:64, qs], piece[0:64, :], Ident,
                            bias=bq[mo][0:64, :],
                        )
                        nc.scalar.activation(
                            qdup[2 * mo + 1][64:128, qs], piece[64:128, :], Ident,
                            bias=bq[mo][64:128, :],
                        )
                    else:
                        nc.vector.tensor_scalar_add(
                            qdup[2 * mo][0:64, qs], piece[0:64, :], bq[mo][0:64, :]
                        )
                        nc.vector.tensor_scalar_add(
                            qdup[2 * mo + 1][64:128, qs], piece[64:128, :],
                            bq[mo][64:128, :],
                        )
                else:
                    mk = mo - 3
                    if act_copy:
                        nc.scalar.activation(
                            kdup[2 * mk][0:64, qs], piece[0:64, :], Ident, bias=0.0
                        )
                        nc.scalar.activation(
                            kdup[2 * mk + 1][64:128, qs], piece[64:128, :], Ident,
                            bias=0.0,
                        )
                    else:
                        nc.vector.tensor_copy(kdup[2 * mk][0:64, qs], piece[0:64, :])
                        nc.vector.tensor_copy(
                            kdup[2 * mk + 1][64:128, qs], piece[64:128, :]
                        )

        def p1_mo(mo):
            # one 128-row stripe of q^T/k^T (= 2 heads' halves), in two
            # 1024-wide pieces through the shared "s" psum ring
            p1_piece(mo, 0)
            p1_piece(mo, 1)

        def dup_heads(hs):
            for hh in hs:
                if hh % 2 == 0:
                    nc.sync.dma_start(out=qdup[hh][64:128, :], in_=qdup[hh][0:64, :])
                    nc.gpsimd.dma_start(out=kdup[hh][64:128, :], in_=kdup[hh][0:64, :])
                else:
                    nc.sync.dma_start(out=qdup[hh][0:64, :], in_=qdup[hh][64:128, :])
                    nc.gpsimd.dma_start(out=kdup[hh][0:64, :], in_=kdup[hh][64:128, :])

        def p2_mt(mt):
            vps = ps.tile([P, C], F32, tag="nd", name="v_ps")
            for k in range(3):
                nc.tensor.matmul(
                    vps[:],
                    xT[k][:, P * mt : P * (mt + 1)],
                    wv[k][:],
                    start=(k == 0),
                    stop=(k == 2),
                )
            # even heads' v -> cols 256a+0, odd heads' -> 256a+192,
            # via two strided casts (ones blocks pre-filled by DMA)
            va = vaug[mt].rearrange("p (a b d) -> p a b d", a=3, b=4, d=D)
            vp = vps.rearrange("p (a c d) -> p a c d", a=3, c=2, d=D)
            nc.vector.tensor_copy(va[:, :, 0, :], vp[:, :, 0, :])
            nc.vector.tensor_copy(va[:, :, 3, :], vp[:, :, 1, :])

        # ---- attention helpers ----
        def emit_s_exp(h, qh, mt):
            s = ps.tile([P, QH], F32, tag="s", name="s")
            for c in range(2):
                qs = slice(QH * qh + 512 * c, QH * qh + 512 * (c + 1))
                cs = slice(512 * c, 512 * (c + 1))
                nc.tensor.matmul(
                    s[:, cs], kdup[h][:, P * mt : P * (mt + 1)], qdup[h][:, qs],
                    start=True, stop=True,
                )
            e = epool.tile([P, QH], BF16, tag="e", name="e")
            nc.scalar.activation(e[:], s[:], Exp)
            return e

        def emit_nd(h, nd, mt, e):
            for c in range(2):
                cs = slice(512 * c, 512 * (c + 1))
                nc.tensor.matmul(
                    nd[:, cs],
                    vaug[mt][:, P * h : P * (h + 1)],
                    e[:, cs],
                    start=(mt == 0), stop=(mt == NMT - 1),
                )

        def normalize(h, qh, nd):
            num_p = slice(0, 64) if h % 2 == 0 else slice(64, 128)
            den_p = slice(64, 128) if h % 2 == 0 else slice(0, 64)
            r = rpool.tile([P, QH], F32, tag="r", name="r")
            nc.vector.reciprocal(r[den_p, :], nd[den_p, :])
            nc.sync.dma_start(out=r[num_p, :], in_=r[den_p, :])
            nc.vector.tensor_mul(
                aT[h // 2][num_p, QH * qh : QH * (qh + 1)],
                nd[num_p, :],
                r[num_p, :],
            )

        def group(h, qh, extras=()):
            # 1-deep software pipeline: s(mt+1) queued on PE before nd(mt);
            # extras are drip-fed prologue chunks filling PE/DVE slack
            extras = list(extras)
            nd = ps.tile([P, QH], F32, tag="nd", name="nd")
            e_prev = emit_s_exp(h, qh, 0)
            for mt in range(1, NMT):
                e_cur = emit_s_exp(h, qh, mt)
                emit_nd(h, nd, mt - 1, e_prev)
                e_prev = e_cur
                if mt % 3 == 0 and extras:
                    extras.pop(0)()
            emit_nd(h, nd, NMT - 1, e_prev)
            for ex in extras:
                ex()
            normalize(h, qh, nd)

        # ---- emission schedule ----
        # vaug ones pattern arrives by DMA (v slots overwritten by p2 casts)
        for mt in range(NMT):
            nc.gpsimd.dma_start(out=vaug[mt][:], in_=ones_e[:])

        # minimal critical prologue for the first scores: q stripe half 0
        # and k stripe m-tiles 0..7 of heads 0/1, dups on the idle sync queue
        p1_piece(0, 0, act_copy=True)
        p1_piece(3, 0, act_copy=True)
        nc.sync.dma_start(out=qdup[0][64:128, 0:QH], in_=qdup[0][0:64, 0:QH])
        nc.sync.dma_start(out=kdup[0][64:128, 0:QH], in_=kdup[0][0:64, 0:QH])

        es0 = [emit_s_exp(0, 0, mt) for mt in range(8)]
        p1_piece(3, 1, tag="nd", act_copy=True)
        nc.sync.dma_start(out=kdup[0][64:128, QH:N], in_=kdup[0][0:64, QH:N])
        es0 += [emit_s_exp(0, 0, mt) for mt in range(8, NMT)]

        for mt in range(NMT):
            p2_mt(mt)

        p1_piece(0, 1, tag="nd")
        nc.sync.dma_start(out=qdup[0][64:128, QH:N], in_=qdup[0][0:64, QH:N])
        nc.sync.dma_start(out=qdup[1][0:64, :], in_=qdup[1][64:128, :])
        nc.gpsimd.dma_start(out=kdup[1][0:64, :], in_=kdup[1][64:128, :])

        # global 1-group-deep pipeline: group g's nd-matmuls interleave with
        # group g+1's scores/exp so the PE queue never drains at boundaries
        seq = [(h, qh) for h in range(H) for qh in range(2)]
        extras_map = {
            2: [lambda: p1_piece(1, 0, tag="nd"), lambda: p1_piece(1, 1, tag="nd")],
            3: [lambda: p1_piece(4, 0, tag="nd"), lambda: p1_piece(4, 1, tag="nd"),
                lambda: dup_heads([2, 3])],
            5: [lambda: p1_piece(2, 0, tag="nd"), lambda: p1_piece(2, 1, tag="nd")],
            6: [lambda: p1_piece(5, 0, tag="nd"), lambda: p1_piece(5, 1, tag="nd"),
                lambda: dup_heads([4, 5])],
        }
        es_prev = es0
        nd_prev = ps.tile([P, QH], F32, tag="nd", name="nd")
        hq_prev = (0, 0)
        for gi in range(1, len(seq)):
            h, qh = seq[gi]
            extras = list(extras_map.get(gi, ()))
            if gi == len(seq) - 1:
                # last group: chase the previous group's nd AND run its own
                # nd one m-tile behind, so the tail after the final exp is
                # just two nd-matmuls + normalize
                nd = ps.tile([P, QH], F32, tag="nd", name="nd")
                e_last = None
                for mt in range(NMT):
                    e_cur = emit_s_exp(h, qh, mt)
                    emit_nd(hq_prev[0], nd_prev, mt, es_prev[mt])
                    if mt > 0:
                        emit_nd(h, nd, mt - 1, e_last)
                    e_last = e_cur
                normalize(hq_prev[0], hq_prev[1], nd_prev)
                emit_nd(h, nd, NMT - 1, e_last)
                normalize(h, qh, nd)
                break
            es_cur = []
            nd_cur = ps.tile([P, QH], F32, tag="nd", name="nd")
            for mt in range(NMT):
                es_cur.append(emit_s_exp(h, qh, mt))
                emit_nd(hq_prev[0], nd_prev, mt, es_prev[mt])
                if mt in (7, 10, 13) and extras:
                    extras.pop(0)()
            for ex in extras:
                ex()
            normalize(hq_prev[0], hq_prev[1], nd_prev)
            es_prev, nd_prev, hq_prev = es_cur, nd_cur, (h, qh)

        # ---- proj: out^T = pwT.T @ aT + bp, through the "s" ring ----
        for mo in range(3):
            for ph in range(2):
                pj = ps.tile([P, QH], F32, tag="s", name="pj")
                for c in range(2):
                    qs = slice(QH * ph + 512 * c, QH * ph + 512 * (c + 1))
                    cs = slice(512 * c, 512 * (c + 1))
                    for k in range(3):
                        nc.tensor.matmul(
                            pj[:, cs],
                            pw[k][:, P * mo : P * (mo + 1)],
                            aT[k][:, qs],
                            start=(k == 0),
                            stop=(k == 2),
                        )
                o = opool.tile([P, QH], F32, tag="o", name="o")
                nc.scalar.activation(o[:], pj[:], Ident, bias=bp[mo][:])
                eng = [nc.sync, nc.gpsimd, nc.scalar][(2 * mo + ph) % 3]
                eng.dma_start(
                    out=out_e[P * mo : P * (mo + 1), QH * ph : QH * (ph + 1)],
                    in_=o[:],
                )

    nc.compile()
    return nc


def _get_nc():
    global _NC
    if _NC is None:
        _NC = _build_nc()
    return _NC


def kernel(x, qkv_w, qkv_b, proj_w, proj_b, h=None, w=None, _trace=False):
    global LAST_RESULT
    x = np.asarray(x, dtype=np.float32)
    qkv_w = np.asarray(qkv_w, dtype=np.float32)
    qkv_b = np.asarray(qkv_b, dtype=np.float32)
    proj_w = np.asarray(proj_w, dtype=np.float32)
    proj_b = np.asarray(proj_b, dtype=np.float32)

    bf16 = ml_dtypes.bfloat16
    # q scale (and the 0.5 for the duplicated-K contraction) folded into
    # Wq/bq; k-bias dropped (softmax shift-invariant); v-bias folded into
    # the proj bias (attention rows sum to 1).
    wqkT = np.concatenate(
        [qkv_w[:C] * (SCALE * 0.5), qkv_w[C : 2 * C]], axis=0
    ).T.astype(bf16).copy()                        # [C, 2C]
    wvT = qkv_w[2 * C :].T.astype(bf16).copy()     # [C, C]
    pwT = proj_w.T.astype(bf16).copy()             # [C, C]
    bq = (qkv_b[:C] * (SCALE * 0.5)).astype(np.float32).reshape(C, 1)
    bp = (proj_b + qkv_b[2 * C :] @ proj_w.T).astype(np.float32).reshape(C, 1)

    vones = np.ones((P, H * P), dtype=bf16)
    common = {"wqkT": wqkT, "wvT": wvT, "pwT": pwT, "bq": bq, "bp": bp,
              "vones": vones}
    in_maps = []
    for i in range(NCORES):
        xT = np.ascontiguousarray(x[i].T).astype(bf16)
        in_maps.append({"xT": xT, **common})

    nc = _get_nc()
    import os as _os

    kw = {}
    if _os.environ.get("KEEP_TMPDIR"):
        kw["tmpdir"] = _os.environ["KEEP_TMPDIR"]
    res = run_bass_kernel_spmd(
        nc, in_maps, core_ids=list(range(NCORES)), trace=_trace, **kw
    )
    LAST_RESULT = res

    out = np.empty((B, N, C), dtype=np.float32)
    for i in range(NCORES):
        out[i] = res.results[i]["out"].T
    return out


if __name__ == "__main__":
    rng = np.random.default_rng(0)
    x = rng.standard_normal((B, N, C), dtype=np.float32)
    s = 1.0 / np.sqrt(C)
    qkv_w = rng.uniform(-s, s, (3 * C, C)).astype(np.float32)
    qkv_b = rng.uniform(-s, s, (3 * C,)).astype(np.float32)
    proj_w = rng.uniform(-s, s, (C, C)).astype(np.float32)
    proj_b = rng.uniform(-s, s, (C,)).astype(np.float32)
    out = kernel(x, qkv_w, qkv_b, proj_w, proj_b, 64, 32)
    print("out", out.shape, out.dtype, float(np.abs(out).mean()))
